# revision 26
# baseline (speedup 1.0000x reference)
"""Multi-head causal self-attention (B=2, S=2048, D=1024, H=16) on 8 TRN2 cores.

Sharding: head-parallel for QKV+attention (core c owns heads {2c, 2c+1}),
token-parallel for the output projection, bridged by four half-batch
AllToAlls of the normalized context (each core ends up with the full
1024-dim context for its 128-token slice of each half-batch) — 8x less
wire traffic than gathering or reduce-scattering partial outputs.

The PE clock gate (HAM) re-throttles to 1.2GHz whenever any ~3.4us window
has an idle gap; the attention kg chain (PE scores -> ACT exp -> DVE mask
-> PE ctx) alone always has such gaps. So the emission engine interleaves
dependency-free "filler" matmul units into the PE queue: batch 1's QKV
projection + V-transposes fill batch 0's attention (fillerA), batch 1's
late projection chunks + batch 0's output projection fill batch 1's
attention (fillerB, demand-flushed so k/v tiles exist before the
attention chunk that reads them). bc/normalize consumers are emitted one
h-slot late so the PE queue never waits on the l-reciprocal chain.

Per core (matmul operands bf16; PSUM accumulation f32):
  stage B: Q^T/K^T/V^T = (x @ W{q,k,v}[:, c-slice] + b)^T   [128, 4096]
  stage C: V^T -> V_aug [tok, 65] tiles (col 64 = ones -> l row)
  stage D: scores^T = K^T.T @ Q^T tiles (PE), exp (ACT, scale=1/8,
           diagonal blocks packed so no masked column is exp'd),
           post-exp multiplicative causal mask (DVE), ctx^T accum (PE)
  stage E: l row -> r = exp(-ln(l)) (ACT, same table as exp), PE
           outer-product broadcast, normalize -> bf16
  stage F: per half-batch: AllToAll ctx, then out = Wo^T-tiles @
           ctx_full + bo for this core's 128 tokens, full Wo

Host: x pre-transposed bf16; weights host-pre-tiled to [p, kt, n] so all
loads are contiguous; output reassembled token-wise from yT2.
"""

import sys

for p in ("/opt/trn_rl_repo", "/root/.axon_site/_ro/trn_rl_repo"):
    if p not in sys.path:
        sys.path.insert(0, p)

from collections import deque

import numpy as np

import bass_rust
import concourse.bass as bass
import concourse.mybir as mybir
from concourse.bass_utils import run_bass_kernel_spmd
from concourse.masks import make_identity
from concourse.tile import TileContext

B, S, D = 2, 2048, 1024
H, DH = 16, 64
T = B * S              # 4096 tokens
NC = 8                 # cores
HG = D // NC           # 128 qkv dims per core (2 heads)
KT_D = D // 128        # 8 contraction tiles over d_model
QC = 512               # q-chunk width
NQC = S // QC          # 4 q-chunks per batch
HB = S // 2            # half-batch tokens (one A2A each)
TPH = HB // NC         # 128 tokens per core per half-batch
INV_SCALE = 1.0 / float(np.sqrt(DH))  # 1/8
F32 = mybir.dt.float32
F32R = mybir.dt.float32r
BF16 = mybir.dt.bfloat16


def _split_waits(nc, max_waits=1):
    """This walrus build accepts one sync-wait per instruction; Tile sometimes
    emits more. Split extras into preceding NoOps on the same engine."""
    n = 0
    for f in nc.m.functions:
        for bb in f.blocks:
            out = []
            for inst in bb.instructions:
                si = getattr(inst, "sync_info", None)
                if si is not None and si.on_wait and len(si.on_wait) > max_waits:
                    waits = list(si.on_wait)
                    head, rest = waits[:-max_waits], waits[-max_waits:]
                    k = 0
                    while head:
                        chunk, head = head[:max_waits], head[max_waits:]
                        out.append(mybir.InstNoOp(
                            name=f"{inst.name}-wsplit-{k}", ins=[], outs=[],
                            engine=inst.engine,
                            sync_info=bass_rust.SyncInfo(on_wait=chunk, on_update=[]),
                        ))
                        k += 1
                    si.on_wait = rest
                    n += 1
                out.append(inst)
            bb.instructions = out
    return n


def build_module():
    nc = bass.Bass()

    # weights arrive host-pre-tiled ([p, kt, n] flattened) so the loads are
    # fully contiguous DMAs instead of 256B-descriptor gather patterns
    xT = nc.dram_tensor("xT", [D, T], BF16, kind="ExternalInput")
    wq = nc.dram_tensor("wq", [128, KT_D * HG], BF16, kind="ExternalInput")
    wk = nc.dram_tensor("wk", [128, KT_D * HG], BF16, kind="ExternalInput")
    wv = nc.dram_tensor("wv", [128, KT_D * HG], BF16, kind="ExternalInput")
    wo = nc.dram_tensor("wo", [128, KT_D * D], BF16, kind="ExternalInput")
    bq = nc.dram_tensor("bq", [HG, 1], F32, kind="ExternalInput")
    bk = nc.dram_tensor("bk", [HG, 1], F32, kind="ExternalInput")
    bv = nc.dram_tensor("bv", [HG, 1], F32, kind="ExternalInput")
    bo = nc.dram_tensor("bo", [128, KT_D], F32, kind="ExternalInput")
    # output: per batch, this core's two 128-token half-batch slices
    yT2 = nc.dram_tensor("yT2", [D, B * 2 * TPH], F32, kind="ExternalOutput")

    # AllToAll buffers per (batch, half): [shard/rank, 128, TPH]
    a2a_in = [[nc.dram_tensor(f"a2i{b}{hf}", [NC, HG, TPH], BF16)
               for hf in range(2)] for b in range(B)]
    a2a_out = [[nc.dram_tensor(f"a2o{b}{hf}", [NC, HG, TPH], BF16)
                for hf in range(2)] for b in range(B)]

    with TileContext(nc) as tc:
        with tc.tile_pool(name="persist", bufs=1) as pp:
            w_sb = {}
            for name, dram in (("wq", wq), ("wk", wk), ("wv", wv)):
                t = pp.tile([128, KT_D, HG], BF16, name=f"{name}_sb", tag=f"{name}_sb")
                nc.sync.dma_start(out=t[:], in_=dram[:].rearrange("p (kt n) -> p kt n", n=HG))
                w_sb[name] = t
            b_sb = {}
            for name, dram in (("bq", bq), ("bk", bk), ("bv", bv)):
                t = pp.tile([HG, 1], F32, name=f"{name}_sb", tag=f"{name}_sb")
                nc.sync.dma_start(out=t[:], in_=dram[:])
                b_sb[name] = t
            # Wo/bo tiles allocated now, loaded at D-phase start: the 2MB
            # load must not compete with stage B's x/w streaming
            wo_sb = pp.tile([128, KT_D, D], BF16, name="wo_sb", tag="wo_sb")
            bo_sb = pp.tile([128, KT_D], F32, name="bo_sb", tag="bo_sb")

            ident_f = pp.tile([128, 128], F32, name="ident_f", tag="ident_f")
            make_identity(nc, ident_f[:])
            ident = pp.tile([128, 128], BF16, name="ident", tag="ident")
            nc.vector.tensor_copy(ident[:], ident_f[:])
            # multiplicative causal mask for a diagonal 128x128 tile of
            # scores^T: keep [r, c] where r <= c (k <= q)
            tri_f = pp.tile([128, 128], F32, name="tri_f", tag="tri_f")
            nc.gpsimd.memset(tri_f[:], 1.0)
            nc.gpsimd.affine_select(
                out=tri_f[:], in_=tri_f[:],
                compare_op=mybir.AluOpType.is_ge, fill=0.0,
                base=0, pattern=[[1, 128]], channel_multiplier=-1,
            )
            tri01 = pp.tile([128, 128], BF16, name="tri01", tag="tri01")
            nc.vector.tensor_copy(tri01[:], tri_f[:])
            # ones row at partition 64 (base partition of the l row)
            ones_f = pp.tile([65, DH], F32, name="ones_f", tag="ones_f")
            nc.vector.memset(ones_f[:], 1.0)
            ones_r = pp.tile([65, DH], F32R, name="ones_r", tag="ones_r")
            nc.vector.tensor_copy(ones_r[:], ones_f[:])
            ones128 = pp.tile([128, B * 2 * (S // 128)], F32, name="ones128",
                              tag="ones128")
            nc.vector.memset(ones128[:], 1.0)

            qkvT = {}
            for name in ("qT", "kT", "vT"):
                qkvT[name] = [pp.tile([128, S], BF16, name=f"{name}{b}", tag=f"{name}{b}")
                              for b in range(B)]

            vaug = pp.tile([128, B * 2, S // 128, DH + 1], BF16, name="vaug", tag="vaug")
            nc.vector.tensor_copy(vaug[:, :, :, DH:DH + 1], ones128[:, :])
            # [65 used partitions, pair, q]; row 64 = l
            ctxu = pp.tile([128, B * 2, S], F32, name="ctxu", tag="ctxu")

            def emit_C4(b, h, idx, pst):
                # V^T -> V_aug transposes for ktiles 4*idx..4*idx+3;
                # pst: [128, >=256] bf16 PSUM region
                pr = b * 2 + h
                for j in range(4):
                    kt = idx * 4 + j
                    nc.tensor.transpose(
                        out=pst[:, j * DH:(j + 1) * DH],
                        in_=qkvT["vT"][b][h * DH:(h + 1) * DH,
                                          kt * 128:(kt + 1) * 128],
                        identity=ident[h * DH:(h + 1) * DH,
                                       h * DH:(h + 1) * DH],
                    )
                nc.vector.tensor_copy(
                    vaug[:, pr, idx * 4:(idx + 1) * 4, 0:DH],
                    pst[:, 0:256],
                )

            # ---------------- stage B+C for batch 0 ----------------
            # 3 accumulator tags x 2 bufs: adjacent 512-token chunks
            # accumulate in parallel banks, so chunk boundaries never wait
            # on the bias-add drains
            with (
                tc.tile_pool(name="xt0_pool", bufs=3) as xt0_pool,
                tc.tile_pool(name="psB", bufs=2, space="PSUM") as psB_pool,
                tc.tile_pool(name="psT", bufs=2, space="PSUM") as psT_pool,
            ):
                for tqg in range(2):
                    t0 = tqg * 1024
                    acc = {}
                    for ch in range(2):
                        for wname in ("wq", "wk", "wv"):
                            acc[(ch, wname)] = psB_pool.tile(
                                [128, 512], F32, name=f"ps{wname}",
                                tag=f"ps{wname}")
                    for kt in range(KT_D):
                        xt = xt0_pool.tile([128, 1024], BF16, name="xt", tag="xt")
                        nc.sync.dma_start(
                            out=xt[:],
                            in_=xT[kt * 128:(kt + 1) * 128, t0:t0 + 1024])
                        for ch in range(2):
                            for wname in ("wq", "wk", "wv"):
                                nc.tensor.matmul(
                                    acc[(ch, wname)][:],
                                    w_sb[wname][:, kt, :],
                                    xt[:, ch * 512:(ch + 1) * 512],
                                    start=(kt == 0), stop=(kt == KT_D - 1),
                                )
                    for ch in range(2):
                        for wname, bname in (("wq", "bq"), ("wk", "bk"),
                                             ("wv", "bv")):
                            nc.vector.tensor_scalar_add(
                                out=qkvT[wname.replace("w", "") + "T"][0][
                                    :, t0 + ch * 512:t0 + (ch + 1) * 512],
                                in0=acc[(ch, wname)][:],
                                scalar1=b_sb[bname][:, 0:1],
                            )
                for h in range(2):
                    for idx in range(4):
                        pst = psT_pool.tile([128, 256], BF16, name="pst", tag="pst")
                        emit_C4(0, h, idx, pst[:])

            # ------- stages D-F + interleaved filler -------
            with (
                tc.tile_pool(name="psS", bufs=2, space="PSUM") as psS_pool,
                tc.tile_pool(name="psC", bufs=1, space="PSUM") as psC_pool,
                tc.tile_pool(name="mps", bufs=1, space="PSUM") as mps_pool,
                tc.tile_pool(name="psBI", bufs=1, space="PSUM") as psBI_pool,
                tc.tile_pool(name="xt_pool", bufs=16) as xt_pool,
                tc.tile_pool(name="exp_pool", bufs=4) as exp_pool,
                tc.tile_pool(name="rpool", bufs=4) as rpool,
                tc.tile_pool(name="cn_pool", bufs=4) as cn_pool,
                tc.tile_pool(name="cf_pool", bufs=4) as cf_pool,
                tc.tile_pool(name="yo_pool", bufs=2) as yo_pool,
            ):
                nc.gpsimd.dma_start(
                    out=wo_sb[:], in_=wo[:].rearrange("p (kt n) -> p kt n", n=D))
                nc.gpsimd.dma_start(out=bo_sb[:], in_=bo[:])

                r_tiles = {}
                cn_tiles = {}
                ctxf_tiles = {}
                pending = []       # deferred bc/normalize slots

                # ---- filler units: dependency-free PE work ----
                xt1_tiles = {}     # tqgroup -> list of 8 tiles

                def u_load(tqg, half):
                    def f():
                        tiles = xt1_tiles.setdefault(tqg, [])
                        for kt in range(half * 4, half * 4 + 4):
                            xt = xt_pool.tile([128, 1024], BF16, name="xt1",
                                              tag="xt1")
                            nc.sync.dma_start(
                                out=xt[:],
                                in_=xT[kt * 128:(kt + 1) * 128,
                                       S + tqg * 1024: S + tqg * 1024 + 1024])
                            tiles.append(xt)
                    return f

                def u_pass(ch, wname, bname, acc_tag):
                    # one projection pass for batch 1's 512-token chunk ch
                    def f():
                        tqg, sub = divmod(ch, 2)
                        acc = psBI_pool.tile([128, 512], F32, name=acc_tag,
                                             tag=acc_tag)
                        tiles = xt1_tiles[tqg]
                        for kt in range(KT_D):
                            nc.tensor.matmul(
                                acc[:],
                                w_sb[wname][:, kt, :],
                                tiles[kt][:, sub * 512:(sub + 1) * 512],
                                start=(kt == 0), stop=(kt == KT_D - 1),
                            )
                        t0 = ch * 512
                        nc.vector.tensor_scalar_add(
                            out=qkvT[wname.replace("w", "") + "T"][1][:, t0:t0 + 512],
                            in0=acc[:],
                            scalar1=b_sb[bname][:, 0:1],
                        )
                    return f

                def u_c1(h, idx):
                    def f():
                        # transpose scratch borrowed from a psS tile (bitcast
                        # the f32 bank region to bf16)
                        ps_t = psS_pool.tile([128, 1024], F32, name="ps_s",
                                             tag="ps_s")
                        emit_C4(1, h, idx, ps_t[:].bitcast(BF16))
                    return f

                def b1_units(tqg):
                    us = [("xt", tqg, u_load(tqg, 0)), ("xt", tqg, u_load(tqg, 1))]
                    for sub in range(2):
                        ch = tqg * 2 + sub
                        for wname, bname, acc in (("wq", "bq", "accA"),
                                                  ("wk", "bk", "accB"),
                                                  ("wv", "bv", "accA")):
                            us.append((wname[1], ch, u_pass(ch, wname, bname, acc)))
                        for h in range(2):
                            us.append(("C", ch, u_c1(h, ch)))
                    return us

                fillerA = deque(b1_units(0))
                fillerB = deque(b1_units(1))

                def flush(filler, pred=None):
                    while filler and (pred is None or pred(filler[0])):
                        filler.popleft()[2]()

                def emit_D(b, qc, h, filler, fill_every):
                    q0 = qc * QC
                    n_kt = q0 // 128 + 4
                    pr = b * 2 + h
                    qT_h = qkvT["qT"][b][h * DH:(h + 1) * DH, :]
                    kT_h = qkvT["kT"][b][h * DH:(h + 1) * DH, :]
                    ps_ctx = psC_pool.tile([128, QC], F32, name="ps_ctx",
                                           tag="ps_ctx")
                    for kg in range(n_kt // 2):
                        ka, kb = 2 * kg, 2 * kg + 1
                        offa = max(0, ka * 128 - q0)
                        offb = max(0, kb * 128 - q0)
                        ps_s = psS_pool.tile([128, 1024], F32, name="ps_s",
                                             tag="ps_s")
                        # kb's block packed at column 512 (width 512-offb):
                        # the exp range [offa:1024-offb] has no dead gap
                        nc.tensor.matmul(
                            ps_s[:, offa:512],
                            kT_h[:, ka * 128:(ka + 1) * 128],
                            qT_h[:, q0 + offa:q0 + 512],
                            start=True, stop=True,
                        )
                        nc.tensor.matmul(
                            ps_s[:, 512:1024 - offb],
                            kT_h[:, kb * 128:(kb + 1) * 128],
                            qT_h[:, q0 + offb:q0 + 512],
                            start=True, stop=True,
                        )
                        ex = exp_pool.tile([128, 1024], BF16, name="ex", tag="ex")
                        nc.scalar.activation(
                            out=ex[:, offa:1024 - offb],
                            in_=ps_s[:, offa:1024 - offb],
                            func=mybir.ActivationFunctionType.Exp,
                            scale=INV_SCALE,
                        )
                        if ka * 128 >= q0:
                            nc.vector.tensor_mul(
                                out=ex[:, offa:offa + 128],
                                in0=ex[:, offa:offa + 128],
                                in1=tri01[:],
                            )
                        if kb * 128 >= q0:
                            nc.vector.tensor_mul(
                                out=ex[:, 512:640],
                                in0=ex[:, 512:640],
                                in1=tri01[:],
                            )
                        nc.tensor.matmul(
                            ps_ctx[0:DH + 1, offa:512],
                            vaug[:, pr, ka, :],
                            ex[:, offa:512],
                            start=(ka == 0), stop=False,
                            skip_group_check=True,
                        )
                        nc.tensor.matmul(
                            ps_ctx[0:DH + 1, offb:512],
                            vaug[:, pr, kb, :],
                            ex[:, 512:1024 - offb],
                            start=False, stop=(kb == n_kt - 1),
                            skip_group_check=True,
                        )
                        if filler and kg % fill_every == fill_every - 1:
                            filler.popleft()[2]()
                    nc.vector.tensor_copy(
                        ctxu[0:DH + 1, pr, q0:q0 + 512],
                        ps_ctx[0:DH + 1, :],
                    )
                    # r = 1/l = exp(-ln(l)): ln/exp share the attention exp's
                    # ACT table; the exp writes f32r (a rounding op, so the
                    # f32r matmul consumer passes BIR verification)
                    ln_f = rpool.tile([65, QC], F32, name="ln_f", tag="ln_f")
                    nc.scalar.activation(
                        out=ln_f[64:65, :], in_=ctxu[64:65, pr, q0:q0 + QC],
                        func=mybir.ActivationFunctionType.Ln)
                    r_t = rpool.tile([65, QC], F32R, name="r_t", tag="r_t")
                    nc.scalar.activation(
                        out=r_t[64:65, :], in_=ln_f[64:65, :],
                        func=mybir.ActivationFunctionType.Exp, scale=-1.0)
                    r_tiles[(b, qc, h)] = r_t

                def emit_bcnorm(b, qc, h):
                    # deferred one h-slot: the PE bc matmul's reciprocal input
                    # is long since ready, so the PE queue never stalls here
                    q0 = qc * QC
                    pr = b * 2 + h
                    if h == 0:
                        cn_tiles[(b, qc)] = cn_pool.tile(
                            [128, QC], BF16, name="cn", tag="cn")
                    cn = cn_tiles[(b, qc)]
                    r_t = r_tiles.pop((b, qc, h))
                    bc = mps_pool.tile([128, QC], F32, name="bc", tag="mps")
                    nc.tensor.matmul(
                        bc[0:DH, :],
                        ones_r[64:65, 0:DH],
                        r_t[64:65, :],
                        start=True, stop=True,
                    )
                    nc.vector.tensor_mul(
                        out=cn[h * DH:(h + 1) * DH, :],
                        in0=ctxu[0:DH, pr, q0:q0 + QC],
                        in1=bc[0:DH, :],
                    )
                    if h == 1:
                        # chunk done: ship to the half-batch A2A input (4
                        # shards of 128 tokens). Sync queue — the gpsimd
                        # queue blocks on in-flight collectives.
                        hf, sub = divmod(qc, 2)
                        nc.sync.dma_start(
                            out=a2a_in[b][hf][:].rearrange("g p n -> p g n")[
                                :, 4 * sub:4 * sub + 4, :],
                            in_=cn[:].rearrange("p (g n) -> p g n", g=4),
                        )
                        del cn_tiles[(b, qc)]

                def emit_a2a(b, hf):
                    nc.gpsimd.collective_compute(
                        "AllToAll",
                        mybir.AluOpType.bypass,
                        ins=[a2a_in[b][hf][:]],
                        outs=[a2a_out[b][hf][:]],
                        replica_groups=[list(range(NC))],
                    )
                    ctxf = cf_pool.tile([128, KT_D, TPH], BF16, name="ctxf",
                                        tag="ctxf")
                    nc.gpsimd.dma_start(
                        out=ctxf[:],
                        in_=a2a_out[b][hf][:].rearrange("kt p n -> p kt n"))
                    ctxf_tiles[(b, hf)] = ctxf

                def u_outproj(b, hf, og4):
                    # 4 out-dim tiles of the (b, half)-batch token-sharded
                    # output projection (shares the mps bank with bc)
                    def f():
                        ctxf = ctxf_tiles[(b, hf)]
                        ps_o = mps_pool.tile([128, QC], F32, name="ps_o",
                                             tag="mps")
                        for sub in range(4):
                            ot = og4 * 4 + sub
                            for kt in range(KT_D):
                                nc.tensor.matmul(
                                    ps_o[:, sub * TPH:(sub + 1) * TPH],
                                    wo_sb[:, kt, ot * 128:(ot + 1) * 128],
                                    ctxf[:, kt, :],
                                    start=(kt == 0), stop=(kt == KT_D - 1),
                                    skip_group_check=True,
                                )
                        yo = yo_pool.tile([128, 4, TPH], F32, name="yo", tag="yo")
                        for sub in range(4):
                            ot = og4 * 4 + sub
                            nc.vector.tensor_scalar_add(
                                out=yo[:, sub, :],
                                in0=ps_o[:, sub * TPH:(sub + 1) * TPH],
                                scalar1=bo_sb[:, ot:ot + 1],
                            )
                        nc.sync.dma_start(
                            out=yT2[og4 * 512:(og4 + 1) * 512,
                                    (2 * b + hf) * TPH:
                                    (2 * b + hf + 1) * TPH].rearrange(
                                "(ot p) n -> p ot n", p=128),
                            in_=yo[:],
                        )
                    return f

                # demand predicates: whether a fillerB unit must run before
                # attention chunk (1, qc) (which reads ktiles < n_kt and
                # qT chunk qc)
                def need(qc):
                    n_kt = qc * 4 + 4
                    def pred(u):
                        kind, idx = u[0], u[1]
                        if kind == "xt":
                            return True   # loads gate everything behind them
                        if kind == "q":
                            return idx <= qc
                        if kind in ("k", "v"):
                            return idx * 4 < n_kt
                        if kind == "C":
                            return idx * 4 < n_kt
                        return False      # out-proj units are never required
                    return pred

                for b in range(B):
                    filler = fillerA if b == 0 else fillerB
                    fe = 3 if b == 0 else 2
                    for qc in range(NQC):
                        if b == 1:
                            flush(fillerB, need(qc))
                        emit_D(b, qc, 0, filler, fe)
                        if pending:
                            pending.pop(0)()
                        emit_D(b, qc, 1, filler, fe)
                        emit_bcnorm(b, qc, 0)
                        pending.append(
                            (lambda bb=b, qq=qc: (
                                emit_bcnorm(bb, qq, 1),
                                emit_a2a(bb, qq // 2) if qq % 2 == 1 else None)))
                    if b == 0:
                        flush(fillerA)
                        while pending:
                            pending.pop(0)()
                        # b0's out-proj units become D(b1) filler
                        for hf in range(2):
                            for og4 in range(2):
                                fillerB.append(("o", 0, u_outproj(0, hf, og4)))
                while pending:
                    pending.pop(0)()
                flush(fillerB)
                # tail: b1 half 0's out-proj overlaps half 1's A2A
                for og4 in range(2):
                    u_outproj(1, 0, og4)()
                for og4 in range(2):
                    u_outproj(1, 1, og4)()

    _split_waits(nc)
    return nc


def _tile_w(w):
    # [D, N] -> [128, KT_D * N]: contraction tile kt on partitions
    w = np.asarray(w)
    n = w.shape[1]
    return np.ascontiguousarray(
        w.reshape(KT_D, 128, n).transpose(1, 0, 2).reshape(128, KT_D * n))


def kernel(x, mask, Wq, bq, Wk, bk, Wv, bv, Wo, bo, trace=False):
    import ml_dtypes
    bf16 = ml_dtypes.bfloat16
    x = np.asarray(x, dtype=np.float32).reshape(T, D)
    xT = np.ascontiguousarray(x.T).astype(bf16)
    Wo_bf = _tile_w(np.asarray(Wo, np.float32)).astype(bf16)
    bo_f = np.ascontiguousarray(
        np.asarray(bo, np.float32).reshape(KT_D, 128).T)
    in_maps = []
    for c in range(NC):
        sl = slice(c * HG, (c + 1) * HG)
        in_maps.append({
            "xT": xT,
            "wq": _tile_w(np.asarray(Wq, np.float32)[:, sl]).astype(bf16),
            "wk": _tile_w(np.asarray(Wk, np.float32)[:, sl]).astype(bf16),
            "wv": _tile_w(np.asarray(Wv, np.float32)[:, sl]).astype(bf16),
            "wo": Wo_bf,
            "bq": np.ascontiguousarray(np.asarray(bq, np.float32)[sl].reshape(HG, 1)),
            "bk": np.ascontiguousarray(np.asarray(bk, np.float32)[sl].reshape(HG, 1)),
            "bv": np.ascontiguousarray(np.asarray(bv, np.float32)[sl].reshape(HG, 1)),
            "bo": bo_f,
        })
    nc = build_module()
    res = run_bass_kernel_spmd(nc, in_maps, core_ids=list(range(NC)), trace=trace)
    out = np.empty((B, S, D), dtype=np.float32)
    for c in range(NC):
        y = res.results[c]["yT2"]  # [D, B*2*TPH]
        for b in range(B):
            for hf in range(2):
                t0 = hf * HB + c * TPH
                col = (2 * b + hf) * TPH
                out[b, t0:t0 + TPH, :] = y[:, col:col + TPH].T
    if trace:
        kernel.last_results = res
    return out.reshape(B, S, D)


# revision 35
# speedup vs baseline: 1.0161x; 1.0161x over previous
"""Multi-head causal self-attention (B=2, S=2048, D=1024, H=16) on 8 TRN2 cores.

Sharding: head-parallel for QKV+attention (core c owns heads {2c, 2c+1}),
token-parallel for the output projection, bridged by four half-batch
AllToAlls of the normalized context (each core ends up with the full
1024-dim context for its 128-token slice of each half-batch) — 8x less
wire traffic than gathering or reduce-scattering partial outputs.

The PE clock gate (HAM) re-throttles to 1.2GHz whenever any ~3.4us window
has an idle gap; the attention kg chain (PE scores -> ACT exp -> DVE mask
-> PE ctx) alone always has such gaps. So the emission engine interleaves
dependency-free "filler" matmul units into the PE queue: batch 1's QKV
projection + V-transposes fill batch 0's attention (fillerA), batch 1's
late projection chunks + batch 0's output projection fill batch 1's
attention (fillerB, demand-flushed so k/v tiles exist before the
attention chunk that reads them). bc/normalize consumers are emitted one
h-slot late so the PE queue never waits on the l-reciprocal chain.

Per core (matmul operands bf16; PSUM accumulation f32):
  stage B: Q^T/K^T/V^T = (x @ W{q,k,v}[:, c-slice] + b)^T   [128, 4096]
  stage C: V^T -> V_aug [tok, 65] tiles (col 64 = ones -> l row)
  stage D: scores^T = K^T.T @ Q^T tiles (PE), exp (ACT, scale=1/8,
           diagonal blocks packed so no masked column is exp'd),
           post-exp multiplicative causal mask (DVE), ctx^T accum (PE)
  stage E: l row -> r = exp(-ln(l)) (ACT, same table as exp), PE
           outer-product broadcast, normalize -> bf16
  stage F: per half-batch: AllToAll ctx, then out = Wo^T-tiles @
           ctx_full + bo for this core's 128 tokens, full Wo

Host: x pre-transposed bf16; weights host-pre-tiled to [p, kt, n] so all
loads are contiguous; output reassembled token-wise from yT2.
"""

import sys

for p in ("/opt/trn_rl_repo", "/root/.axon_site/_ro/trn_rl_repo"):
    if p not in sys.path:
        sys.path.insert(0, p)

from collections import deque

import numpy as np

import bass_rust
import concourse.bass as bass
import concourse.mybir as mybir
from concourse.bass_utils import run_bass_kernel_spmd
from concourse.masks import make_identity
from concourse.tile import TileContext

B, S, D = 2, 2048, 1024
H, DH = 16, 64
T = B * S              # 4096 tokens
NC = 8                 # cores
HG = D // NC           # 128 qkv dims per core (2 heads)
KT_D = D // 128        # 8 contraction tiles over d_model
QC = 512               # q-chunk width
NQC = S // QC          # 4 q-chunks per batch
TPC = S // NC          # 256 tokens per core per batch (out-proj sharding)
INV_SCALE = 1.0 / float(np.sqrt(DH))  # 1/8
F32 = mybir.dt.float32
F32R = mybir.dt.float32r
BF16 = mybir.dt.bfloat16


def _split_waits(nc, max_waits=1):
    """This walrus build accepts one sync-wait per instruction; Tile sometimes
    emits more. Split extras into preceding NoOps on the same engine."""
    n = 0
    for f in nc.m.functions:
        for bb in f.blocks:
            out = []
            for inst in bb.instructions:
                si = getattr(inst, "sync_info", None)
                if si is not None and si.on_wait and len(si.on_wait) > max_waits:
                    waits = list(si.on_wait)
                    head, rest = waits[:-max_waits], waits[-max_waits:]
                    k = 0
                    while head:
                        chunk, head = head[:max_waits], head[max_waits:]
                        out.append(mybir.InstNoOp(
                            name=f"{inst.name}-wsplit-{k}", ins=[], outs=[],
                            engine=inst.engine,
                            sync_info=bass_rust.SyncInfo(on_wait=chunk, on_update=[]),
                        ))
                        k += 1
                    si.on_wait = rest
                    n += 1
                out.append(inst)
            bb.instructions = out
    return n


def build_module():
    nc = bass.Bass()

    # weights arrive host-pre-tiled ([p, kt, n] flattened) so the loads are
    # fully contiguous DMAs instead of 256B-descriptor gather patterns
    xT = nc.dram_tensor("xT", [D, T], BF16, kind="ExternalInput")
    wq = nc.dram_tensor("wq", [128, KT_D * HG], BF16, kind="ExternalInput")
    wk = nc.dram_tensor("wk", [128, KT_D * HG], BF16, kind="ExternalInput")
    wv = nc.dram_tensor("wv", [128, KT_D * HG], BF16, kind="ExternalInput")
    wo = nc.dram_tensor("wo", [128, KT_D * D], BF16, kind="ExternalInput")
    bq = nc.dram_tensor("bq", [HG, 1], F32, kind="ExternalInput")
    bk = nc.dram_tensor("bk", [HG, 1], F32, kind="ExternalInput")
    bv = nc.dram_tensor("bv", [HG, 1], F32, kind="ExternalInput")
    bo = nc.dram_tensor("bo", [128, KT_D], F32, kind="ExternalInput")
    # output: this core's TPC tokens of each batch, all D dims
    yT2 = nc.dram_tensor("yT2", [D, B * TPC], F32, kind="ExternalOutput")

    # AllToAll buffers per batch: [shard/rank, 128, TPC]
    a2a_in = [nc.dram_tensor(f"a2i{b}", [NC, HG, TPC], BF16) for b in range(B)]
    a2a_out = [nc.dram_tensor(f"a2o{b}", [NC, HG, TPC], BF16) for b in range(B)]

    with TileContext(nc) as tc:
        with tc.tile_pool(name="persist", bufs=1) as pp:
            w_sb = {}
            for name, dram in (("wq", wq), ("wk", wk), ("wv", wv)):
                t = pp.tile([128, KT_D, HG], BF16, name=f"{name}_sb", tag=f"{name}_sb")
                nc.sync.dma_start(out=t[:], in_=dram[:].rearrange("p (kt n) -> p kt n", n=HG))
                w_sb[name] = t
            b_sb = {}
            for name, dram in (("bq", bq), ("bk", bk), ("bv", bv)):
                t = pp.tile([HG, 1], F32, name=f"{name}_sb", tag=f"{name}_sb")
                nc.sync.dma_start(out=t[:], in_=dram[:])
                b_sb[name] = t
            # Wo/bo tiles allocated now, loaded at D-phase start: the 2MB
            # load must not compete with stage B's x/w streaming
            wo_sb = pp.tile([128, KT_D, D], BF16, name="wo_sb", tag="wo_sb")
            bo_sb = pp.tile([128, KT_D], F32, name="bo_sb", tag="bo_sb")

            ident_f = pp.tile([128, 128], F32, name="ident_f", tag="ident_f")
            make_identity(nc, ident_f[:])
            ident = pp.tile([128, 128], BF16, name="ident", tag="ident")
            nc.vector.tensor_copy(ident[:], ident_f[:])
            # multiplicative causal mask for a diagonal 128x128 tile of
            # scores^T: keep [r, c] where r <= c (k <= q)
            tri_f = pp.tile([128, 128], F32, name="tri_f", tag="tri_f")
            nc.gpsimd.memset(tri_f[:], 1.0)
            nc.gpsimd.affine_select(
                out=tri_f[:], in_=tri_f[:],
                compare_op=mybir.AluOpType.is_ge, fill=0.0,
                base=0, pattern=[[1, 128]], channel_multiplier=-1,
            )
            tri01 = pp.tile([128, 128], BF16, name="tri01", tag="tri01")
            nc.vector.tensor_copy(tri01[:], tri_f[:])
            # ones row at partition 64 (base partition of the l row)
            ones_f = pp.tile([65, DH], F32, name="ones_f", tag="ones_f")
            nc.vector.memset(ones_f[:], 1.0)
            # bf16 so the bc matmul's weight load uses the fast path
            ones_r = pp.tile([65, DH], BF16, name="ones_r", tag="ones_r")
            nc.vector.tensor_copy(ones_r[:], ones_f[:])
            ones128 = pp.tile([128, B * 2 * (S // 128)], F32, name="ones128",
                              tag="ones128")
            nc.vector.memset(ones128[:], 1.0)

            qkvT = {}
            for name in ("qT", "kT", "vT"):
                qkvT[name] = [pp.tile([128, S], BF16, name=f"{name}{b}", tag=f"{name}{b}")
                              for b in range(B)]

            vaug = pp.tile([128, B * 2, S // 128, DH + 1], BF16, name="vaug", tag="vaug")
            nc.vector.tensor_copy(vaug[:, :, :, DH:DH + 1], ones128[:, :])
            # [65 used partitions, pair, q]; row 64 = l
            ctxu = pp.tile([128, B * 2, S], F32, name="ctxu", tag="ctxu")

            def emit_C4(b, h, idx, pst):
                # V^T -> V_aug transposes for ktiles 4*idx..4*idx+3;
                # pst: [128, >=256] bf16 PSUM region
                pr = b * 2 + h
                for j in range(4):
                    kt = idx * 4 + j
                    nc.tensor.transpose(
                        out=pst[:, j * DH:(j + 1) * DH],
                        in_=qkvT["vT"][b][h * DH:(h + 1) * DH,
                                          kt * 128:(kt + 1) * 128],
                        identity=ident[h * DH:(h + 1) * DH,
                                       h * DH:(h + 1) * DH],
                    )
                nc.vector.tensor_copy(
                    vaug[:, pr, idx * 4:(idx + 1) * 4, 0:DH],
                    pst[:, 0:256],
                )

            # ---------------- stage B+C for batch 0 ----------------
            # 3 accumulator tags x 2 bufs: adjacent 512-token chunks
            # accumulate in parallel banks, so chunk boundaries never wait
            # on the bias-add drains
            with (
                tc.tile_pool(name="xt0_pool", bufs=5) as xt0_pool,
                tc.tile_pool(name="psB", bufs=2, space="PSUM") as psB_pool,
                tc.tile_pool(name="psT", bufs=2, space="PSUM") as psT_pool,
            ):
                for tqg in range(2):
                    t0 = tqg * 1024
                    acc = {}
                    for ch in range(2):
                        for wname in ("wq", "wk", "wv"):
                            acc[(ch, wname)] = psB_pool.tile(
                                [128, 512], F32, name=f"ps{wname}",
                                tag=f"ps{wname}")
                    for kt in range(KT_D):
                        xt = xt0_pool.tile([128, 1024], BF16, name="xt", tag="xt")
                        nc.sync.dma_start(
                            out=xt[:],
                            in_=xT[kt * 128:(kt + 1) * 128, t0:t0 + 1024])
                        for ch in range(2):
                            for wname in ("wq", "wk", "wv"):
                                nc.tensor.matmul(
                                    acc[(ch, wname)][:],
                                    w_sb[wname][:, kt, :],
                                    xt[:, ch * 512:(ch + 1) * 512],
                                    start=(kt == 0), stop=(kt == KT_D - 1),
                                )
                    for ch in range(2):
                        for wname, bname in (("wq", "bq"), ("wk", "bk"),
                                             ("wv", "bv")):
                            nc.vector.tensor_scalar_add(
                                out=qkvT[wname.replace("w", "") + "T"][0][
                                    :, t0 + ch * 512:t0 + (ch + 1) * 512],
                                in0=acc[(ch, wname)][:],
                                scalar1=b_sb[bname][:, 0:1],
                            )
                for h in range(2):
                    for idx in range(4):
                        pst = psT_pool.tile([128, 256], BF16, name="pst", tag="pst")
                        emit_C4(0, h, idx, pst[:])

            # ------- stages D-F + interleaved filler -------
            with (
                tc.tile_pool(name="psS", bufs=2, space="PSUM") as psS_pool,
                tc.tile_pool(name="psC", bufs=1, space="PSUM") as psC_pool,
                tc.tile_pool(name="mps", bufs=1, space="PSUM") as mps_pool,
                tc.tile_pool(name="psBI", bufs=1, space="PSUM") as psBI_pool,
                tc.tile_pool(name="xt_pool", bufs=16) as xt_pool,
                tc.tile_pool(name="exp_pool", bufs=4) as exp_pool,
                tc.tile_pool(name="rpool", bufs=4) as rpool,
                tc.tile_pool(name="cn_pool", bufs=4) as cn_pool,
                tc.tile_pool(name="cf_pool", bufs=4) as cf_pool,
                tc.tile_pool(name="yo_pool", bufs=2) as yo_pool,
            ):
                nc.gpsimd.dma_start(
                    out=wo_sb[:], in_=wo[:].rearrange("p (kt n) -> p kt n", n=D))
                nc.gpsimd.dma_start(out=bo_sb[:], in_=bo[:])

                r_tiles = {}
                cn_tiles = {}
                ctxf_tiles = {}
                pending = []       # deferred bc/normalize slots

                # ---- filler units: dependency-free PE work ----
                xt1_tiles = {}     # tqgroup -> list of 8 tiles

                def u_load(tqg, half):
                    def f():
                        tiles = xt1_tiles.setdefault(tqg, [])
                        for kt in range(half * 4, half * 4 + 4):
                            xt = xt_pool.tile([128, 1024], BF16, name="xt1",
                                              tag="xt1")
                            nc.sync.dma_start(
                                out=xt[:],
                                in_=xT[kt * 128:(kt + 1) * 128,
                                       S + tqg * 1024: S + tqg * 1024 + 1024])
                            tiles.append(xt)
                    return f

                def u_pass(ch, wname, bname, acc_tag):
                    # one projection pass for batch 1's 512-token chunk ch
                    def f():
                        tqg, sub = divmod(ch, 2)
                        acc = psBI_pool.tile([128, 512], F32, name=acc_tag,
                                             tag=acc_tag)
                        tiles = xt1_tiles[tqg]
                        for kt in range(KT_D):
                            nc.tensor.matmul(
                                acc[:],
                                w_sb[wname][:, kt, :],
                                tiles[kt][:, sub * 512:(sub + 1) * 512],
                                start=(kt == 0), stop=(kt == KT_D - 1),
                            )
                        t0 = ch * 512
                        nc.vector.tensor_scalar_add(
                            out=qkvT[wname.replace("w", "") + "T"][1][:, t0:t0 + 512],
                            in0=acc[:],
                            scalar1=b_sb[bname][:, 0:1],
                        )
                    return f

                def u_c1(h, idx):
                    def f():
                        # transpose scratch borrowed from a psS tile (bitcast
                        # the f32 bank region to bf16)
                        ps_t = psS_pool.tile([128, 1024], F32, name="ps_s",
                                             tag="ps_s")
                        emit_C4(1, h, idx, ps_t[:].bitcast(BF16))
                    return f

                def ch_units(ch):
                    us = []
                    if ch % 2 == 0:
                        tqg = ch // 2
                        us += [("xt", tqg, u_load(tqg, 0)),
                               ("xt", tqg, u_load(tqg, 1))]
                    for wname, bname, acc in (("wq", "bq", "accA"),
                                              ("wk", "bk", "accB"),
                                              ("wv", "bv", "accA")):
                        us.append((wname[1], ch, u_pass(ch, wname, bname, acc)))
                    for h in range(2):
                        us.append(("C", ch, u_c1(h, ch)))
                    return us

                # chunks 0-2 of batch 1's projection fill batch 0's
                # attention; chunk 3 (+ batch 0's out-proj, appended later)
                # fills batch 1's
                fillerA = deque(ch_units(0) + ch_units(1) + ch_units(2))
                fillerB = deque(ch_units(3))

                def flush(filler, pred=None):
                    while filler and (pred is None or pred(filler[0])):
                        filler.popleft()[2]()

                def emit_D(b, qc, h, filler, fill_every):
                    q0 = qc * QC
                    n_kt = q0 // 128 + 4
                    pr = b * 2 + h
                    qT_h = qkvT["qT"][b][h * DH:(h + 1) * DH, :]
                    kT_h = qkvT["kT"][b][h * DH:(h + 1) * DH, :]
                    ps_ctx = psC_pool.tile([128, QC], F32, name="ps_ctx",
                                           tag="ps_ctx")
                    for kg in range(n_kt // 2):
                        ka, kb = 2 * kg, 2 * kg + 1
                        offa = max(0, ka * 128 - q0)
                        offb = max(0, kb * 128 - q0)
                        ps_s = psS_pool.tile([128, 1024], F32, name="ps_s",
                                             tag="ps_s")
                        # kb's block packed at column 512 (width 512-offb):
                        # the exp range [offa:1024-offb] has no dead gap
                        nc.tensor.matmul(
                            ps_s[:, offa:512],
                            kT_h[:, ka * 128:(ka + 1) * 128],
                            qT_h[:, q0 + offa:q0 + 512],
                            start=True, stop=True,
                        )
                        nc.tensor.matmul(
                            ps_s[:, 512:1024 - offb],
                            kT_h[:, kb * 128:(kb + 1) * 128],
                            qT_h[:, q0 + offb:q0 + 512],
                            start=True, stop=True,
                        )
                        ex = exp_pool.tile([128, 1024], BF16, name="ex", tag="ex")
                        nc.scalar.activation(
                            out=ex[:, offa:1024 - offb],
                            in_=ps_s[:, offa:1024 - offb],
                            func=mybir.ActivationFunctionType.Exp,
                            scale=INV_SCALE,
                        )
                        if ka * 128 >= q0:
                            nc.vector.tensor_mul(
                                out=ex[:, offa:offa + 128],
                                in0=ex[:, offa:offa + 128],
                                in1=tri01[:],
                            )
                        if kb * 128 >= q0:
                            nc.vector.tensor_mul(
                                out=ex[:, 512:640],
                                in0=ex[:, 512:640],
                                in1=tri01[:],
                            )
                        nc.tensor.matmul(
                            ps_ctx[0:DH + 1, offa:512],
                            vaug[:, pr, ka, :],
                            ex[:, offa:512],
                            start=(ka == 0), stop=False,
                            skip_group_check=True,
                        )
                        nc.tensor.matmul(
                            ps_ctx[0:DH + 1, offb:512],
                            vaug[:, pr, kb, :],
                            ex[:, 512:1024 - offb],
                            start=False, stop=(kb == n_kt - 1),
                            skip_group_check=True,
                        )
                        if filler and kg % fill_every == fill_every - 1:
                            filler.popleft()[2]()
                    nc.vector.tensor_copy(
                        ctxu[0:DH + 1, pr, q0:q0 + 512],
                        ps_ctx[0:DH + 1, :],
                    )
                    # r = 1/l = exp(-ln(l)): ln/exp share the attention exp's
                    # ACT table; the exp writes f32r (a rounding op, so the
                    # f32r matmul consumer passes BIR verification)
                    ln_f = rpool.tile([65, QC], F32, name="ln_f", tag="ln_f")
                    nc.scalar.activation(
                        out=ln_f[64:65, :], in_=ctxu[64:65, pr, q0:q0 + QC],
                        func=mybir.ActivationFunctionType.Ln)
                    r_t = rpool.tile([65, QC], BF16, name="r_t", tag="r_t")
                    nc.scalar.activation(
                        out=r_t[64:65, :], in_=ln_f[64:65, :],
                        func=mybir.ActivationFunctionType.Exp, scale=-1.0)
                    r_tiles[(b, qc, h)] = r_t

                def emit_bcnorm(b, qc, h):
                    # deferred one h-slot: the PE bc matmul's reciprocal input
                    # is long since ready, so the PE queue never stalls here
                    q0 = qc * QC
                    pr = b * 2 + h
                    if h == 0:
                        cn_tiles[(b, qc)] = cn_pool.tile(
                            [128, QC], BF16, name="cn", tag="cn")
                    cn = cn_tiles[(b, qc)]
                    r_t = r_tiles.pop((b, qc, h))
                    bc = mps_pool.tile([128, QC], F32, name="bc", tag="mps")
                    nc.tensor.matmul(
                        bc[0:DH, :],
                        ones_r[64:65, 0:DH],
                        r_t[64:65, :],
                        start=True, stop=True,
                    )
                    nc.vector.tensor_mul(
                        out=cn[h * DH:(h + 1) * DH, :],
                        in0=ctxu[0:DH, pr, q0:q0 + QC],
                        in1=bc[0:DH, :],
                    )
                    if h == 1:
                        # chunk done: ship to the A2A input buffer (token
                        # groups 2qc, 2qc+1). Sync queue — the gpsimd queue
                        # blocks on in-flight collectives.
                        nc.sync.dma_start(
                            out=a2a_in[b][:].rearrange("g p n -> p g n")[
                                :, 2 * qc:2 * qc + 2, :],
                            in_=cn[:].rearrange("p (g n) -> p g n", g=2),
                        )
                        del cn_tiles[(b, qc)]

                def emit_a2a(b):
                    nc.gpsimd.collective_compute(
                        "AllToAll",
                        mybir.AluOpType.bypass,
                        ins=[a2a_in[b][:]],
                        outs=[a2a_out[b][:]],
                        replica_groups=[list(range(NC))],
                    )
                    ctxf = cf_pool.tile([128, KT_D, TPC], BF16, name="ctxf",
                                        tag="ctxf")
                    nc.gpsimd.dma_start(
                        out=ctxf[:],
                        in_=a2a_out[b][:].rearrange("kt p n -> p kt n"))
                    ctxf_tiles[b] = ctxf

                def u_outproj(b, og):
                    # two out-dim tiles of batch b's token-sharded output
                    # projection (shares the mps bank with bc)
                    def f():
                        ctxf = ctxf_tiles[b]
                        ps_o = mps_pool.tile([128, QC], F32, name="ps_o",
                                             tag="mps")
                        for sub in range(2):
                            ot = og * 2 + sub
                            for kt in range(KT_D):
                                nc.tensor.matmul(
                                    ps_o[:, sub * TPC:(sub + 1) * TPC],
                                    wo_sb[:, kt, ot * 128:(ot + 1) * 128],
                                    ctxf[:, kt, :],
                                    start=(kt == 0), stop=(kt == KT_D - 1),
                                    skip_group_check=True,
                                )
                        yo = yo_pool.tile([128, 2, TPC], F32, name="yo", tag="yo")
                        for sub in range(2):
                            ot = og * 2 + sub
                            nc.vector.tensor_scalar_add(
                                out=yo[:, sub, :],
                                in0=ps_o[:, sub * TPC:(sub + 1) * TPC],
                                scalar1=bo_sb[:, ot:ot + 1],
                            )
                        nc.sync.dma_start(
                            out=yT2[og * 256:(og + 1) * 256,
                                    b * TPC:(b + 1) * TPC].rearrange(
                                "(ot p) n -> p ot n", p=128),
                            in_=yo[:],
                        )
                    return f

                # demand predicates: whether a fillerB unit must run before
                # attention chunk (1, qc) (which reads ktiles < n_kt and
                # qT chunk qc)
                def need(qc):
                    n_kt = qc * 4 + 4
                    def pred(u):
                        kind, idx = u[0], u[1]
                        if kind == "xt":
                            return True   # loads gate everything behind them
                        if kind == "q":
                            return idx <= qc
                        if kind in ("k", "v"):
                            return idx * 4 < n_kt
                        if kind == "C":
                            return idx * 4 < n_kt
                        return False      # out-proj units are never required
                    return pred

                for b in range(B):
                    filler = fillerA if b == 0 else fillerB
                    for qc in range(NQC):
                        if b == 1:
                            flush(fillerB, need(qc))
                        # out-proj filler units only pop in the last chunk,
                        # well after batch 0's A2A has completed
                        fe = 1 if (b == 1 and qc == 3) else 2
                        emit_D(b, qc, 0, filler, fe)
                        if pending:
                            pending.pop(0)()
                        emit_D(b, qc, 1, filler, fe)
                        emit_bcnorm(b, qc, 0)
                        pending.append(
                            (lambda bb=b, qq=qc: (
                                emit_bcnorm(bb, qq, 1),
                                emit_a2a(bb) if qq == NQC - 1 else None)))
                    if b == 0:
                        flush(fillerA)
                        while pending:
                            pending.pop(0)()
                        # b0's out-proj units become D(b1) late filler
                        for og in range(KT_D // 2):
                            fillerB.append(("o", 0, u_outproj(0, og)))
                while pending:
                    pending.pop(0)()
                flush(fillerB)
                for og in range(KT_D // 2):
                    u_outproj(1, og)()

    _split_waits(nc)
    return nc


def _tile_w(w):
    # [D, N] -> [128, KT_D * N]: contraction tile kt on partitions
    w = np.asarray(w)
    n = w.shape[1]
    return np.ascontiguousarray(
        w.reshape(KT_D, 128, n).transpose(1, 0, 2).reshape(128, KT_D * n))


def kernel(x, mask, Wq, bq, Wk, bk, Wv, bv, Wo, bo, trace=False):
    import ml_dtypes
    bf16 = ml_dtypes.bfloat16
    x = np.asarray(x, dtype=np.float32).reshape(T, D)
    xT = np.ascontiguousarray(x.T).astype(bf16)
    Wo_bf = _tile_w(np.asarray(Wo, np.float32)).astype(bf16)
    bo_f = np.ascontiguousarray(
        np.asarray(bo, np.float32).reshape(KT_D, 128).T)
    in_maps = []
    for c in range(NC):
        sl = slice(c * HG, (c + 1) * HG)
        in_maps.append({
            "xT": xT,
            "wq": _tile_w(np.asarray(Wq, np.float32)[:, sl]).astype(bf16),
            "wk": _tile_w(np.asarray(Wk, np.float32)[:, sl]).astype(bf16),
            "wv": _tile_w(np.asarray(Wv, np.float32)[:, sl]).astype(bf16),
            "wo": Wo_bf,
            "bq": np.ascontiguousarray(np.asarray(bq, np.float32)[sl].reshape(HG, 1)),
            "bk": np.ascontiguousarray(np.asarray(bk, np.float32)[sl].reshape(HG, 1)),
            "bv": np.ascontiguousarray(np.asarray(bv, np.float32)[sl].reshape(HG, 1)),
            "bo": bo_f,
        })
    nc = build_module()
    res = run_bass_kernel_spmd(nc, in_maps, core_ids=list(range(NC)), trace=trace)
    out = np.empty((B, S, D), dtype=np.float32)
    for c in range(NC):
        y = res.results[c]["yT2"]  # [D, B*TPC]
        for b in range(B):
            out[b, c * TPC:(c + 1) * TPC, :] = y[:, b * TPC:(b + 1) * TPC].T
    if trace:
        kernel.last_results = res
    return out.reshape(B, S, D)
